# revision 35
# baseline (speedup 1.0000x reference)
"""Distributed causal self-attention kernel for 8 TRN2 NeuronCores.

Problem (hardcoded): B=4, T=2048, C=1024, H=16 heads, D=64 head dim, fp32.
  y = softmax(causal(x Wq^T (x Wk^T)^T / sqrt(D))) (x Wv^T) Wp^T + biases

Sharding: data-parallel over B (4 groups) x tensor-parallel over heads
(2 groups of 8 heads).  Core c handles batch c//2, head-group c%2.  Each
core computes a partial output projection y_partial = O_g @ Wp[:,cols_g]^T;
the host sums the two partials of each batch pair (the 2-way all-reduce of
the sharding hint) and adds bp.

Per-core kernel:
  phase 1: project Q/K/V from DMA'd x^T / W^T chunks, cast to bf16 (full
           PE rate like f32r but ~half the PE power draw, so HAM holds
           the 2.4GHz clock instead of bouncing to 1.2GHz as it does for
           sustained fp32 matmuls).  512-wide t blocks keep every matmul
           at the 512-column max so the 128-row LDWEIGHTS always hides
           under the previous matmul's stream.  Q is evicted (DVE, bias
           fused) straight into zero-padded per-(pair,half) bf16 stores
           so the S matmuls contract K=128; K^T packed bf16; V bf16 with
           a ones column at [64] so each half's O matmul accumulates the
           softmax denominator in the PSUM row under its 64 data rows.
  phase 2: qt-outer / pair-inner flash attention.  Per 128-key chunk:
           two S^T matmuls (bf16), one 3D-AP Exp on ScalarE (both
           halves, scale=1/8 fused, bf16 out), causal mask via a
           gpsimd.affine_select restricted to the 128-column diagonal
           strip, and the O^T accumulation delayed by two chunks so the
           in-order PE never waits on the exp.  The delay pipeline runs
           continuously across pair AND qt boundaries (no flush).
           Normalization is fully off ScalarE: 1/denominator via a
           3-op Newton-Raphson on the DVE (int32 bitcast magic-seed,
           one NR step fused with scalar_tensor_tensor; produces the
           NEGATED reciprocal in bf16), K=1 ones matmul broadcast on
           the PE, sign restored in the DVE PSUM eviction, DVE
           multiplies into O^T.  (ScalarE keeps only the chunk exps;
           hw has no divide/reciprocal off the activation tables.)
  phase 3: per qt (woven into the next qt's chunk stream):
           y[t,:] = O^T.T @ Wp_g^T in bf16, DVE eviction, DMA out.
           Projection PSUM comes from the S pool so the two-deep pso
           rotation is never disturbed.
"""

import ml_dtypes
import numpy as np

import concourse.bass as bass
import concourse.mybir as mybir
from concourse.tile import TileContext
from concourse.bass_utils import run_bass_kernel_spmd

F32 = mybir.dt.float32
BF16 = mybir.dt.bfloat16
I32 = mybir.dt.int32
RCP_MAGIC = 0x7EF311C3
AF = mybir.ActivationFunctionType
ALU = mybir.AluOpType

P = 128          # partitions
T = 2048         # sequence length
C = 1024         # model dim
D = 64           # head dim
HG = 8           # heads per core
J = HG * D       # per-core projection width (512)
CC = C // P      # contraction chunks over model dim (8)
JC = J // P      # j chunks (4)
NT = T // P      # 128-row t tiles (16)
TBS = 512        # t block size for x^T staging
NTB = T // TBS   # t blocks (4)
NQ = T // 512    # 512-wide query tiles (4)
NPAIR = HG // 2  # head pairs (4); pair p covers heads 2p, 2p+1

_CACHE = {}


def _split_excess_waits(nc):
    """Walrus in this container only accepts 1 sync-wait on CTRL-queue
    instructions (Drain etc.).  Hoist excess waits onto preceding nops on
    the same engine queue (program order makes this equivalent)."""
    n = 0
    for f in nc.m.functions:
        for bb in f.blocks:
            out = []
            for inst in bb.instructions:
                si = inst.sync_info
                limit = 1
                if si is not None and si.on_wait and len(si.on_wait) > limit:
                    waits = list(si.on_wait)
                    excess, keep = waits[:-limit], waits[-limit:]
                    for ci in range(0, len(excess), limit):
                        n += 1
                        out.append(mybir.InstNoOp(
                            name=f"waitsplit_{n}", opcode="nop", engine=inst.engine,
                            sync_info=mybir.SyncInfo(
                                on_wait=excess[ci:ci + limit], on_update=[]),
                        ))
                    inst.sync_info = mybir.SyncInfo(
                        on_wait=keep, on_update=list(si.on_update))
                out.append(inst)
            bb.instructions = out


def _build():
    nc = bass.Bass()
    # host passes x and the weight shards pre-transposed (layout marshaling
    # done during sharding): xt = x[b].T, w?t = W?[rows].T, wpt = Wp[:,cols].T
    # inputs come host-pre-shuffled into device-native layouts so every
    # DMA reads one contiguous multi-KB segment per partition (1KB
    # rearrange segments cap HBM throughput well below peak)
    xt_in = nc.dram_tensor("xt", [P, NTB, CC, TBS], BF16, kind="ExternalInput")
    wq_in = nc.dram_tensor("wqt", [P, CC, J], BF16, kind="ExternalInput")
    wk_in = nc.dram_tensor("wkt", [P, CC, J], BF16, kind="ExternalInput")
    wv_in = nc.dram_tensor("wvt", [P, CC, J], BF16, kind="ExternalInput")
    wp_in = nc.dram_tensor("wpt", [P, JC, C], BF16, kind="ExternalInput")
    bq_in = nc.dram_tensor("bq", [J], F32, kind="ExternalInput")
    bk_in = nc.dram_tensor("bk", [J], F32, kind="ExternalInput")
    bv_in = nc.dram_tensor("bv", [J], F32, kind="ExternalInput")
    # partial outputs leave in bf16: the host sums the two TP halves in
    # fp32 anyway, and half-size y stores cut DMA tail and HBM pressure
    y_out = nc.dram_tensor("y", [T, C], BF16, kind="ExternalOutput")

    with TileContext(nc) as tc:
        with tc.tile_pool(name="persist", bufs=1) as persist:
            # Q padded per (pair, half): the other head's 64 partitions are
            # zero so S can contract the full 128-row pair block.
            qp_t = persist.tile([P, NPAIR, 2, T], BF16, tag="qp")
            kt_t = persist.tile([P, JC, T], BF16, tag="kt")     # K^T packed
            # V rows per (t-tile, head): [v0..v63, 1]; the ones column makes
            # PSUM row 64 of each half's O accumulation the softmax
            # denominator.
            v_t = persist.tile([P, NT, HG, D + 1], BF16, tag="v")
            ot_r = persist.tile([P, JC, T], BF16, tag="ot")     # O^T normalized
            bq_sb = persist.tile([P, JC], F32, tag="bq")
            bk_sb = persist.tile([P, JC], F32, tag="bk")
            bv_sb = persist.tile([1, J], F32, tag="bv")
            bv_bf = persist.tile([1, J], BF16, tag="bvbf")
            bv_bc = persist.tile([P, J], F32, tag="bvbc")
            ones_bf = persist.tile([P, P], BF16, tag="ones_bf")

            # constants (gpsimd queue; PE warmup only needs ones_bf so it
            # is emitted first).  The qp dead-half zeros and V ones
            # columns also go here: gpsimd is idle while the first DMAs
            # land, and nothing else touches those slices in phase 1.
            nc.vector.memset(ones_bf[:], 1.0)
            nc.gpsimd.memset(qp_t[0:D, :, 1, :], 0.0)
            nc.gpsimd.memset(qp_t[D:P, :, 0, :], 0.0)
            nc.gpsimd.memset(v_t[:, :, :, D:D + 1], 1.0)
            nc.sync.dma_start(bq_sb[:], bq_in.rearrange("(o p) -> p o", p=P))
            nc.sync.dma_start(bk_sb[:], bk_in.rearrange("(o p) -> p o", p=P))
            nc.sync.dma_start(bv_sb[:], bv_in[None, :])

            # ---------------- phase 1: QKV projections ----------------------
            with (
                tc.tile_pool(name="xt", bufs=2) as xt_pool,
                tc.tile_pool(name="wt", bufs=1) as wt_pool,
                tc.tile_pool(name="ps_mm", bufs=4, space="PSUM") as ps_mm,
            ):
                # HAM warm-up: keep the PE streaming while the first DMAs
                # land so the clock is ramped when projections start.
                ps_warm = ps_mm.tile([P, TBS], F32, tag="mm", name="ps_warm")
                for _ in range(28):
                    nc.tensor.matmul(ps_warm[:, 0:P], lhsT=ones_bf[:],
                                     rhs=ones_bf[:], start=True, stop=True)

                # bv broadcast to all partitions via bf16 K=1 matmul
                nc.vector.tensor_copy(bv_bf[:], bv_sb[:])
                ps_bv = ps_mm.tile([P, J], F32, tag="mm", name="ps_bv")
                nc.tensor.matmul(ps_bv[:], lhsT=ones_bf[0:1, :],
                                 rhs=bv_bf[:], start=True, stop=True)
                nc.vector.tensor_copy(bv_bc[:], ps_bv[:])

                # weights + x^T tiles: raw fp32 DMA into a small staging
                # tile, then ScalarE / gpsimd cast to bf16.  Emission order
                # sets DMA priority: wq and the first x block first, wk/wv
                # behind them.
                wt_q = wt_pool.tile([P, CC, J], BF16, tag="wq", name="wt_q")
                wt_k = wt_pool.tile([P, CC, J], BF16, tag="wk", name="wt_k")
                wt_v = wt_pool.tile([P, CC, J], BF16, tag="wv", name="wt_v")

                # weights and x arrive from the host already in bf16:
                # half the HBM bytes of fp32 and zero cast instructions.
                # x blocks issue on the Activation DMA queue, weights on
                # the SP queue, so the issues overlap; the x DMA for block
                # tb+1 is issued a full block ahead.
                def issue_x(tb, split=False):
                    xt = xt_pool.tile([P, CC, TBS], BF16, tag="xt",
                                      name=f"xt_{tb}")
                    if split:  # quarters so compute starts on the first
                        for qq in range(4):
                            nc.scalar.dma_start(
                                xt[:, 2 * qq:2 * qq + 2, :],
                                xt_in[:, tb, 2 * qq:2 * qq + 2, :])
                    else:
                        nc.scalar.dma_start(xt[:], xt_in[:, tb, :, :])
                    return xt

                xt_next = issue_x(0, split=True)
                for qq in range(4):
                    nc.sync.dma_start(wt_q[:, 2 * qq:2 * qq + 2, :],
                                      wq_in[:, 2 * qq:2 * qq + 2, :])
                nc.sync.dma_start(wt_k[:], wk_in[:, :, :])
                nc.sync.dma_start(wt_v[:], wv_in[:, :, :])

                for tb in range(NTB):
                    xt = xt_next
                    if tb + 1 < NTB:
                        xt_next = issue_x(tb + 1)
                    tbs = slice(tb * TBS, (tb + 1) * TBS)

                    # Q^T / K^T [j, t] per j-chunk (pair)
                    for name, wt, bias in (("q", wt_q, bq_sb), ("k", wt_k, bk_sb)):
                        for jc in range(JC):
                            psq = ps_mm.tile([P, TBS], F32, tag="mm",
                                             name=f"ps_{name}_{tb}_{jc}")
                            for cc in range(CC):
                                nc.tensor.matmul(
                                    psq[:],
                                    lhsT=wt[:, cc, jc * P:(jc + 1) * P],
                                    rhs=xt[:, cc, :],
                                    start=(cc == 0), stop=(cc == CC - 1))
                            if name == "q":
                                nc.vector.tensor_scalar_add(
                                    qp_t[0:D, jc, 0, tbs], psq[0:D, :],
                                    bias[0:D, jc:jc + 1])
                                nc.vector.tensor_scalar_add(
                                    qp_t[D:P, jc, 1, tbs], psq[D:P, :],
                                    bias[D:P, jc:jc + 1])
                            else:
                                nc.vector.tensor_scalar_add(
                                    kt_t[:, jc, tbs], psq[:],
                                    bias[:, jc:jc + 1])

                    # V[t, j] (+ bias broadcast over t)
                    for sub in range(TBS // P):
                        tt = tb * (TBS // P) + sub
                        psv = ps_mm.tile([P, J], F32, tag="mm",
                                         name=f"ps_v_{tt}")
                        for cc in range(CC):
                            nc.tensor.matmul(
                                psv[:],
                                lhsT=xt[:, cc, sub * P:(sub + 1) * P],
                                rhs=wt_v[:, cc, :],
                                start=(cc == 0), stop=(cc == CC - 1))
                        nc.vector.tensor_tensor(
                            v_t[:, tt, :, 0:D],
                            psv.rearrange("p (h d) -> p h d", h=HG),
                            bv_bc.rearrange("p (h d) -> p h d", h=HG),
                            ALU.add)


            # ---------------- phases 2+3 -----------------------------------
            with (
                tc.tile_pool(name="wpt", bufs=1) as wpt_pool,
                tc.tile_pool(name="e", bufs=6) as e_pool,
                tc.tile_pool(name="rc", bufs=2) as rc_pool,
                tc.tile_pool(name="tmp", bufs=2) as tmp_pool,
                tc.tile_pool(name="yout", bufs=2) as y_pool,
                tc.tile_pool(name="ps_s", bufs=2, space="PSUM") as ps_s,
                tc.tile_pool(name="ps_o", bufs=2, space="PSUM") as ps_o,
            ):
                # Wp^T (host pre-transposed, already bf16): one DMA
                wpt = wpt_pool.tile([P, JC, C], BF16, tag="wpt")
                nc.sync.dma_start(wpt[:], wp_in[:, :, :])

                def emit_o(pend):
                    pso, pair, qt, kc, nk, e, delta = pend
                    for half in range(2):
                        h = pair * 2 + half
                        nc.tensor.matmul(
                            pso[0:D + 1, half, delta:],
                            lhsT=v_t[:, kc, h, :],
                            rhs=e[:, half, delta:],
                            start=(kc == 0), stop=(kc == nk - 1))

                RW = slice(D, D + 1)  # denominator row (partition 64)

                def emit_seed(pend):
                    # r0 = bitcast_f32(MAGIC - bitcast_i32(d)): classic
                    # reciprocal seed, straight off the PSUM row
                    pso, pair, qt, kc, nk, e, delta = pend
                    r0 = rc_pool.tile([D + 1, 2, 512], F32, tag="r0",
                                      name=f"r0_{qt}_{pair}")
                    nc.vector.tensor_scalar(
                        r0[RW].bitcast(I32), pso[RW].bitcast(I32),
                        RCP_MAGIC, -1, ALU.subtract, ALU.mult)
                    return r0

                def emit_t(pend, r0):
                    pso, pair, qt, kc, nk, e, delta = pend
                    t0 = rc_pool.tile([D + 1, 2, 512], F32, tag="t0",
                                      name=f"t0_{qt}_{pair}")
                    nc.vector.tensor_tensor(t0[RW], pso[RW], r0[RW], ALU.mult)
                    return t0

                def emit_s1(pend, r0, t0):
                    # s1 = (t-2)*r0 = -(1/d) after one NR step; bf16 out
                    pso, pair, qt, kc, nk, e, delta = pend
                    s1 = rc_pool.tile([D + 1, 2, 512], BF16, tag="s1",
                                      name=f"s1_{qt}_{pair}")
                    nc.vector.scalar_tensor_tensor(
                        s1[RW], t0[RW], 2.0, r0[RW], ALU.subtract, ALU.mult)
                    return s1

                drain_mode = [False]

                def emit_bcast(pend, s1):
                    # K=1 bf16 matmul broadcast of -1/denom to 64 partitions
                    # (in the drain, psb takes the free ps_o buffer so the
                    # early-started projection tiles keep both ps_s bufs)
                    pso, pair, qt, kc, nk, e, delta = pend
                    pool, tg = (ps_o, "o") if drain_mode[0] else (ps_s, "s")
                    psb = pool.tile([P, 2, 512], F32, tag=tg,
                                    name=f"psb_{qt}_{pair}")
                    for half in range(2):
                        nc.tensor.matmul(
                            psb[0:D, half, :], lhsT=ones_bf[D:D + 1, 0:D],
                            rhs=s1[RW.start:RW.stop, half, :],
                            start=True, stop=True)
                    return psb

                def emit_norm(pend, psb):
                    # one PSUM operand max per DVE op: stage bc in SBUF,
                    # then DVE multiplies into normalized O^T
                    pso, pair, qt, kc, nk, e, delta = pend
                    qs = slice(qt * 512, (qt + 1) * 512)
                    bc = tmp_pool.tile([D, 2, 512], F32, tag="bc",
                                       name=f"bc_{qt}_{pair}")
                    # eviction restores the sign of the NR reciprocal
                    nc.vector.tensor_scalar_mul(bc[:], psb[0:D, :, :], -1.0)
                    nc.vector.tensor_tensor(
                        ot_r[0:D, pair, qs], pso[0:D, 0, :],
                        bc[:, 0, :], ALU.mult)
                    # normalized upper half staged in SBUF, then a
                    # cross-partition DMA into O^T rows 64:128
                    tmp = tmp_pool.tile([D, 512], BF16, tag="tmp",
                                        name=f"tmp_{qt}_{pair}")
                    nc.vector.tensor_tensor(
                        tmp[:], pso[0:D, 1, :], bc[:, 1, :], ALU.mult)
                    nc.sync.dma_start(ot_r[D:P, pair, qs], tmp[:])

                psy_open = {}

                def phase3_part(tt, nh, jcs, psy, q=None):
                    ts = slice(tt * P, (tt + 1) * P)
                    for jc in jcs:
                        nc.tensor.matmul(
                            psy[:, nh, :],
                            lhsT=ot_r[:, jc, ts],
                            rhs=wpt[:, jc, nh * 512:(nh + 1) * 512],
                            start=(jc == 0), stop=(jc == JC - 1))
                    if jcs[-1] == JC - 1 and nh == 1:
                        ytile = y_pool.tile([P, C], BF16, tag="y",
                                            name=f"y_{tt}")
                        nc.vector.tensor_copy(
                            ytile.rearrange("p (n q) -> p n q", n=2), psy[:])
                        (q or nc.sync).dma_start(y_out[ts, :], ytile[:])

                def phase3_half(tt, nh):
                    # half a projection tile per action: a 0.9us PE burst
                    # fits the S stream's per-chunk slack, a 1.7us one
                    # does not.  proj PSUM comes from the S pool: same
                    # tile shape, and the pso rotation stays undisturbed.
                    if nh == 0:
                        psy_open[tt] = ps_s.tile([P, 2, 512], F32, tag="s",
                                                 name=f"psy_{tt}")
                    psy = psy_open[tt]
                    phase3_part(tt, nh, list(range(JC)), psy)
                    if nh == 1:
                        del psy_open[tt]

                from collections import deque
                pendq = deque()  # chunks awaiting their O matmuls (depth 2)
                actions = []     # [countdown, fn]: deferred norm/phase-3
                                 # work woven into later chunks

                def tick():
                    fire = []
                    for a in actions:
                        a[0] -= 1
                        if a[0] <= 0:
                            fire.append(a)
                    for a in fire:
                        actions.remove(a)
                        a[1]()

                def fire_o(pend):
                    emit_o(pend)
                    if pend[3] == pend[4] - 1:  # last chunk of its pair
                        state = {}

                        def do_seed(p=pend, s=state):
                            s["r0"] = emit_seed(p)

                        def do_t(p=pend, s=state):
                            s["t0"] = emit_t(p, s["r0"])

                        def do_s1(p=pend, s=state):
                            s["s1"] = emit_s1(p, s["r0"], s["t0"])

                        def do_bcast(p=pend, s=state):
                            s["psb"] = emit_bcast(p, s["s1"])

                        def do_norm(p=pend, s=state):
                            emit_norm(p, s["psb"])

                        # deeper slack for the PE-side bcast/mults when the
                        # next pairs are long enough (qt=0's 4-chunk pairs
                        # need the tight schedule so the 2-deep pso
                        # rotation is respected)
                        last = pend[2] == NQ - 1 and pend[1] == NPAIR - 1
                        cds = (4, 5) if (pend[4] == 4 or last) else (5, 7)
                        actions.append([1, do_seed])
                        actions.append([2, do_t])
                        actions.append([3, do_s1])
                        actions.append([cds[0], do_bcast])
                        actions.append([cds[1], do_norm])

                for qt in range(NQ):
                    nk = (qt + 1) * 4
                    for pair in range(NPAIR):
                        pso = ps_o.tile([P, 2, 512], F32, tag="o",
                                        name=f"pso_{qt}_{pair}")
                        for kc in range(nk):
                            # columns q < delta of this chunk are fully
                            # masked; skip them in S, exp and O.
                            delta = max(0, (kc - qt * 4) * P)
                            ks = slice(kc * P, (kc + 1) * P)
                            pss = ps_s.tile([P, 2, 512], F32, tag="s",
                                            name=f"pss_{qt}_{pair}_{kc}")
                            for half in range(2):
                                nc.tensor.matmul(
                                    pss[:, half, delta:],
                                    lhsT=kt_t[:, pair, ks],
                                    rhs=qp_t[:, pair, half,
                                             qt * 512 + delta:(qt + 1) * 512],
                                    start=True, stop=True)
                            e = e_pool.tile([P, 2, 512], BF16, tag="e",
                                            name=f"e_{qt}_{pair}_{kc}")
                            nc.scalar.activation(
                                e[:, :, delta:], pss[:, :, delta:],
                                AF.Exp, scale=0.125)
                            if kc >= qt * 4:  # diagonal: mask the 128-col strip
                                nc.gpsimd.affine_select(
                                    out=e[:, :, delta:delta + P],
                                    in_=e[:, :, delta:delta + P],
                                    compare_op=ALU.is_ge, fill=0.0,
                                    base=0, channel_multiplier=-1,
                                    pattern=[[0, 2], [1, P]])
                            tick()
                            # short pairs (qt<=1) get a 2-deep O delay so
                            # their norm/psy weave lands earlier; long
                            # pairs keep 3 chunks of exp slack
                            if len(pendq) >= (2 if nk <= 8 else 3):
                                fire_o(pendq.popleft())
                            pendq.append((pso, pair, qt, kc, nk, e, delta))
                    # qt done: the pending O matmuls, the last pair's
                    # normalize and this qt's output projection are all
                    # woven into the next qt's chunk stream (no flush).
                    # countdown 11 keeps these strictly after the last
                    # pair's norm mults (tick 9 worst case) in every qt.
                    # The last qt's tiles are emitted by the drain instead,
                    # overlapped with the final pair's norm chain.
                    if qt != NQ - 1:
                        actions.extend(
                            [11 + i,
                             (lambda t, n: lambda: phase3_half(t, n))(tt, nh)]
                            for i, (tt, nh) in enumerate(
                                (t, n) for t in range(qt * 4, qt * 4 + 4)
                                for n in range(2)))

                # drain: fire remaining pending chunks, then overlap the
                # last pair's DVE norm chain with the final projection
                # tiles' jc0-2 matmuls (which need only the already-normed
                # pairs); jc3 and the stores follow the norm.  Drain y
                # stores go out on the idle Activation DMA queue.
                while pendq:
                    fire_o(pendq.popleft())
                drain_mode[0] = True
                acts = {cd: fn for cd, fn in actions}
                assert len(acts) == 5, sorted(acts)
                acts[1](); acts[2](); acts[3]()      # seed, t, s1 (DVE)
                dts = list(range((NQ - 1) * 4, (NQ - 1) * 4 + 4))
                dpsy = {}
                for tt in dts[:2]:
                    dpsy[tt] = ps_s.tile([P, 2, 512], F32, tag="s",
                                         name=f"psy_{tt}")
                    for nh in range(2):
                        phase3_part(tt, nh, [0, 1, 2], dpsy[tt])
                acts[4](); acts[5]()                 # bcast (PE), mults (DVE)
                for tt in dts[:2]:
                    for nh in range(2):
                        phase3_part(tt, nh, [3], dpsy[tt], q=nc.scalar)
                for tt in dts[2:]:
                    psy = ps_s.tile([P, 2, 512], F32, tag="s",
                                    name=f"psy_{tt}")
                    for nh in range(2):
                        phase3_part(tt, nh, [0, 1, 2, 3], psy, q=nc.scalar)

    _split_excess_waits(nc)
    return nc


def _get_nc():
    if "nc" not in _CACHE:
        _CACHE["nc"] = _build()
    return _CACHE["nc"]


def kernel(x, Wq, bq, Wk, bk, Wv, bv, Wp, bp, **_unused):
    x = np.ascontiguousarray(np.asarray(x, dtype=np.float32))
    Wq = np.asarray(Wq, dtype=np.float32)
    Wk = np.asarray(Wk, dtype=np.float32)
    Wv = np.asarray(Wv, dtype=np.float32)
    Wp = np.asarray(Wp, dtype=np.float32)
    bq = np.asarray(bq, dtype=np.float32)
    bk = np.asarray(bk, dtype=np.float32)
    bv = np.asarray(bv, dtype=np.float32)
    bp = np.asarray(bp, dtype=np.float32)

    nc = _get_nc()
    in_maps = []
    for c in range(8):
        b, g = c // 2, c % 2
        js = slice(g * J, (g + 1) * J)
        bf = ml_dtypes.bfloat16

        def shuf(w):  # [C', F] -> [P, C'//P, F] with partition innermost
            cc, f = w.shape[0] // P, w.shape[1]
            return np.ascontiguousarray(
                w.reshape(cc, P, f).transpose(1, 0, 2).astype(bf))

        xs = x[b].T.reshape(CC, P, NTB, TBS).transpose(1, 2, 0, 3)
        in_maps.append({
            "xt": np.ascontiguousarray(xs.astype(bf)),
            "wqt": shuf(Wq[js, :].T),
            "wkt": shuf(Wk[js, :].T),
            "wvt": shuf(Wv[js, :].T),
            "wpt": shuf(Wp[:, js].T),
            "bq": np.ascontiguousarray(bq[js]),
            "bk": np.ascontiguousarray(bk[js]),
            "bv": np.ascontiguousarray(bv[js]),
        })
    res = run_bass_kernel_spmd(nc, in_maps, list(range(8)))
    out = np.empty((4, T, C), dtype=np.float32)
    for b in range(4):
        out[b] = (np.asarray(res.results[2 * b]["y"]).astype(np.float32)
                  + np.asarray(res.results[2 * b + 1]["y"]).astype(np.float32)
                  + bp)
    return out
